# revision 4
# baseline (speedup 1.0000x reference)
"""AutomatonPELayer kernel for 8 Trainium2 NeuronCores.

Math: pe[j] = T^j @ x0 (j = 0..L-1), out = pe @ W.T + b, with T orthogonal
[128,128], L = 131072, embed dim 512.

Strategy (sequence-sharded, per the spec hint):
- Host (float64, ~20 small matmuls): build the first 128-column block
  X[:, i] = T^i x0, the block-stride powers (T^(128 g)).T for g = 1..G,
  the anchor stride (T^(128 G)).T, and each core's carry T^(16384 m) so
  core m starts its chunk at row m*16384.
- Device (per core, identical program): 128 blocks of 128 rows each.
  Blocks advance by B_{k+1} = T^128 B_k; to keep the serial chain short
  the kernel keeps an "anchor" every G=8 blocks (A_{j+1} = T^(1024) A_j)
  and derives the 7 in-between blocks independently from the anchor.
  Each block k: embed matmul out = B_k.T @ W.T (PSUM), add bias (DVE,
  PSUM->SBUF), DMA to DRAM. Per-core output is 16384x512 f32 (33.5 MB),
  so the kernel is bound by the HBM write (~93 us at 358 GB/s).
"""

import sys

if "/opt/trn_rl_repo" not in sys.path:
    sys.path.insert(0, "/opt/trn_rl_repo")

import numpy as np

L = 131072
S = 128  # num states (= partition dim = contraction dim)
E = 512  # embed dim
NCORES = 8
CHUNK = L // NCORES  # 16384 rows per core
BLOCKS = CHUNK // S  # 128 blocks of 128 rows per core
G = 8  # blocks per anchor group
GROUPS = BLOCKS // G  # 16

_prog_cache = {}


def _build_program():
    if "nc" in _prog_cache:
        return _prog_cache["nc"]

    import concourse.bass as bass
    import concourse.tile as tile
    from concourse import mybir

    def _split_multi_waits(nc):
        """This walrus build accepts only ONE sync-wait per instruction
        (setupSyncWait: 'Too many sync wait commands'). Tile attaches the
        full wait list to the consuming instruction; hoist all but the
        last wait onto single-wait NoOps placed immediately before it on
        the same engine, preserving per-engine program order."""
        uid = 0
        for fn in nc.m.functions:
            for bb in fn.blocks:
                new = []
                changed = False
                for inst in bb.instructions:
                    si = inst.sync_info
                    waits = list(si.on_wait) if si is not None else []
                    if len(waits) > 1:
                        changed = True
                        for w in waits[:-1]:
                            nop = mybir.InstNoOp(
                                name=f"splitw_{uid}",
                                engine=inst.engine,
                                sync_info=mybir.SyncInfo(on_wait=[w], on_update=[]),
                                bass_nofuse=True,
                            )
                            uid += 1
                            new.append(nop)
                        si.on_wait = [waits[-1]]
                    new.append(inst)
                if changed:
                    bb.instructions = new

    f32 = mybir.dt.float32
    nc = bass.Bass("TRN2", target_bir_lowering=False, debug=False, num_devices=NCORES)

    # Per-core inputs. s0 differs per core; the rest are replicated.
    s0 = nc.dram_tensor("s0", [S, S], f32, kind="ExternalInput").ap()
    tgt = nc.dram_tensor("tgt", [S, S], f32, kind="ExternalInput").ap()
    tsteps = nc.dram_tensor("tsteps", [S, G - 1, S], f32, kind="ExternalInput").ap()
    wt = nc.dram_tensor("wt", [S, E], f32, kind="ExternalInput").ap()
    bias = nc.dram_tensor("bias", [128, E], f32, kind="ExternalInput").ap()
    out = nc.dram_tensor("out", [CHUNK, E], f32, kind="ExternalOutput").ap()
    out_v = out.rearrange("(nb p) e -> nb p e", p=S)  # [BLOCKS, 128, E]

    with tile.TileContext(nc) as tc:
        with (
            tc.tile_pool(name="singles", bufs=1) as singles,
            tc.tile_pool(name="anchors", bufs=2) as anchors,
            tc.tile_pool(name="bpool", bufs=4) as bpool,
            tc.tile_pool(name="opool", bufs=4) as opool,
            tc.tile_pool(name="pe_psum", bufs=4, space="PSUM") as pe_psum,
            tc.tile_pool(name="pr_psum", bufs=3, space="PSUM") as pr_psum,
        ):
            wt_t = singles.tile([S, E], f32)
            nc.sync.dma_start(out=wt_t, in_=wt)
            b_t = singles.tile([128, E], f32)
            nc.sync.dma_start(out=b_t, in_=bias)
            tgt_t = singles.tile([S, S], f32)
            nc.sync.dma_start(out=tgt_t, in_=tgt)
            ts_t = singles.tile([S, G - 1, S], f32)
            nc.sync.dma_start(out=ts_t, in_=tsteps)
            a_t = anchors.tile([S, S], f32)
            nc.sync.dma_start(out=a_t, in_=s0)

            for j in range(GROUPS):
                a_next = None
                if j + 1 < GROUPS:
                    # Advance the anchor first: it heads the serial chain.
                    pr = pr_psum.tile([S, S], f32)
                    nc.tensor.matmul(pr, tgt_t, a_t, start=True, stop=True)
                    a_next = anchors.tile([S, S], f32)
                    nc.scalar.copy(out=a_next, in_=pr)
                bs = [a_t]
                for g in range(1, G):
                    pr = pr_psum.tile([S, S], f32)
                    nc.tensor.matmul(
                        pr, ts_t[:, g - 1, :], a_t, start=True, stop=True
                    )
                    b_sb = bpool.tile([S, S], f32)
                    nc.scalar.copy(out=b_sb, in_=pr)
                    bs.append(b_sb)
                for g in range(G):
                    pe = pe_psum.tile([S, E], f32)
                    nc.tensor.matmul(pe, bs[g], wt_t, start=True, stop=True)
                    o_t = opool.tile([S, E], f32)
                    nc.vector.tensor_add(o_t, pe, b_t)
                    nc.sync.dma_start(out=out_v[j * G + g], in_=o_t)
                if a_next is not None:
                    a_t = a_next

    _split_multi_waits(nc)
    _prog_cache["nc"] = nc
    return nc


def _host_precompute(pos_initial, pos_transition):
    """float64 host prep: per-core starting blocks + power matrices."""
    T = np.asarray(pos_transition, np.float64)
    x0 = np.asarray(pos_initial, np.float64).reshape(S)

    # X[:, i] = T^i x0 for i = 0..127 (exact sequential, f64)
    X = np.empty((S, S), np.float64)
    v = x0.copy()
    X[:, 0] = v
    for i in range(1, S):
        v = T @ v
        X[:, i] = v

    # T^128 by repeated squaring
    T128 = T.copy()
    for _ in range(7):
        T128 = T128 @ T128

    # T^(128 g) for g = 0..G
    Tp = [np.eye(S)]
    for g in range(1, G + 1):
        Tp.append(Tp[-1] @ T128)
    TG = Tp[G]  # T^(128 G) = T^1024

    # (T^(128 g)).T stacked: tsteps[:, g-1, :] = (T^(128 g)).T
    tsteps = np.ascontiguousarray(
        np.stack([Tp[g].T for g in range(1, G)], axis=1)
    ).astype(np.float32)
    tgt = np.ascontiguousarray(TG.T).astype(np.float32)

    # per-core carry: T^(CHUNK m); CHUNK = 1024 * 16 so square TG 4 times
    Tchunk = TG.copy()
    for _ in range(4):
        Tchunk = Tchunk @ Tchunk  # T^(1024*16) = T^16384
    s0s = []
    C = np.eye(S)
    for _ in range(NCORES):
        s0s.append(np.ascontiguousarray(C @ X).astype(np.float32))
        C = Tchunk @ C
    return s0s, tgt, tsteps


def kernel(sentence_len, pos_initial, pos_transition, W, b):
    from concourse.bass_utils import run_bass_kernel_spmd

    assert int(sentence_len) == L, f"kernel hardcodes L={L}, got {sentence_len}"
    W = np.asarray(W, np.float32)
    b = np.asarray(b, np.float32)

    s0s, tgt, tsteps = _host_precompute(pos_initial, pos_transition)
    wt = np.ascontiguousarray(W.T)  # [S, E]
    bias = np.ascontiguousarray(np.broadcast_to(b[None, :], (128, E)))

    nc = _build_program()
    in_maps = [
        {"s0": s0s[m], "tgt": tgt, "tsteps": tsteps, "wt": wt, "bias": bias}
        for m in range(NCORES)
    ]
    res = run_bass_kernel_spmd(nc, in_maps, core_ids=list(range(NCORES)))
    return np.concatenate([res.results[m]["out"] for m in range(NCORES)], axis=0)


# revision 7
# speedup vs baseline: 1.2686x; 1.2686x over previous
"""AutomatonPELayer kernel for 8 Trainium2 NeuronCores.

Math: pe[j] = T^j @ x0 (j = 0..L-1), out = pe @ W.T + b, with T orthogonal
[128,128], L = 131072, embed dim 512, fp32.

Strategy (sequence-sharded):
- The output chunk of rows [128k, 128k+128) is B_k.T @ W.T where
  B_k = T^(128k) @ X and X = [x0, T x0, ..., T^127 x0]. Using
  B_{jG+g} = M_g A_j (A_j = T^(128 G j) X the "anchor" of group j,
  M_g = T^(128 g)):   out_block(j,g) = A_j.T @ (M_g.T W.T).
- Host (float64): per-core anchors A_j (16 per core, advancing by
  T^1024; core m offset by T^(16384 m)) and the 8 stride-folded weight
  matrices Wg = M_g.T @ W.T. So the device does ONLY 512-wide embed
  matmuls (fp32r: 1 PE cycle/column), a PSUM->SBUF copy, and the
  output DMA. Per-core output is 16384x512 f32 (33.5 MB) => the kernel
  rides the HBM-write roofline (~94 us at 358 GB/s per core).
- b is folded in on the host only if nonzero (it is zero in this
  problem's setup_inputs); the device path is a pure GEMM.
"""

import sys

if "/opt/trn_rl_repo" not in sys.path:
    sys.path.insert(0, "/opt/trn_rl_repo")

import numpy as np

L = 131072
S = 128  # num states (= partition dim = contraction dim)
E = 512  # embed dim
NCORES = 8
CHUNK = L // NCORES  # 16384 rows per core
BLOCKS = CHUNK // S  # 128 blocks of 128 rows per core
G = 8  # blocks per anchor group
GROUPS = BLOCKS // G  # 16 anchors per core

_prog_cache = {}


def _split_multi_waits(nc, mybir):
    """This walrus build accepts only ONE sync-wait per instruction
    (setupSyncWait: 'Too many sync wait commands'). Tile attaches the
    full wait list to the consuming instruction; hoist all but the
    last wait onto single-wait NoOps placed immediately before it on
    the same engine, preserving per-engine program order."""
    uid = 0
    for fn in nc.m.functions:
        for bb in fn.blocks:
            new = []
            changed = False
            for inst in bb.instructions:
                si = inst.sync_info
                waits = list(si.on_wait) if si is not None else []
                if len(waits) > 1:
                    changed = True
                    for w in waits[:-1]:
                        nop = mybir.InstNoOp(
                            name=f"splitw_{uid}",
                            engine=inst.engine,
                            sync_info=mybir.SyncInfo(on_wait=[w], on_update=[]),
                            bass_nofuse=True,
                        )
                        uid += 1
                        new.append(nop)
                    si.on_wait = [waits[-1]]
                new.append(inst)
            if changed:
                bb.instructions = new


def _build_program():
    if "nc" in _prog_cache:
        return _prog_cache["nc"]

    import concourse.bass as bass
    import concourse.tile as tile
    from concourse import mybir

    f32 = mybir.dt.float32
    f32r = mybir.dt.float32r
    nc = bass.Bass("TRN2", target_bir_lowering=False, debug=False, num_devices=NCORES)

    # anchors differ per core; wgs replicated. float32r = same bits as f32
    # on the host side; tags the PE's fast single-pass fp32 matmul path.
    anchors = nc.dram_tensor("anchors", [GROUPS, S, S], f32r, kind="ExternalInput").ap()
    wgs = nc.dram_tensor("wgs", [G, S, E], f32r, kind="ExternalInput").ap()
    out = nc.dram_tensor("out", [CHUNK, E], f32, kind="ExternalOutput").ap()
    out_v = out.rearrange("(nb p) e -> nb p e", p=S)  # [BLOCKS, 128, E]

    with tile.TileContext(nc) as tc:
        with (
            tc.tile_pool(name="singles", bufs=1) as singles,
            tc.tile_pool(name="opool", bufs=8) as opool,
            tc.tile_pool(name="psum", bufs=8, space="PSUM") as psum,
        ):
            anch_t = singles.tile([S, GROUPS, S], f32r)
            nc.sync.dma_start(out=anch_t, in_=anchors.rearrange("j s i -> s j i"))
            wgs_t = singles.tile([S, G, E], f32r)
            nc.sync.dma_start(out=wgs_t, in_=wgs.rearrange("g s e -> s g e"))

            for j in range(GROUPS):
                for g in range(G):
                    pe = psum.tile([S, E], f32)
                    nc.tensor.matmul(
                        pe,
                        anch_t[:, j, :],
                        wgs_t[:, g, :],
                        start=True,
                        stop=True,
                    )
                    o_t = opool.tile([S, E], f32)
                    nc.vector.tensor_copy(o_t, pe)
                    nc.sync.dma_start(out=out_v[j * G + g], in_=o_t)

    _split_multi_waits(nc, mybir)
    _prog_cache["nc"] = nc
    return nc


def _host_precompute(pos_initial, pos_transition, W):
    """float64 host prep: per-core anchor blocks + stride-folded weights."""
    T = np.asarray(pos_transition, np.float64)
    x0 = np.asarray(pos_initial, np.float64).reshape(S)
    W64 = np.asarray(W, np.float64)

    # X[:, i] = T^i x0 for i = 0..127 (exact sequential, f64)
    X = np.empty((S, S), np.float64)
    v = x0.copy()
    X[:, 0] = v
    for i in range(1, S):
        v = T @ v
        X[:, i] = v

    # T^128 by repeated squaring
    T128 = T.copy()
    for _ in range(7):
        T128 = T128 @ T128

    # M_g = T^(128 g) for g = 0..G
    Tp = [np.eye(S)]
    for g in range(1, G + 1):
        Tp.append(Tp[-1] @ T128)
    TG = Tp[G]  # T^(128 G) = T^1024

    # Wg = M_g.T @ W.T  -> [G, S, E]
    wgs = np.stack([np.ascontiguousarray(Tp[g].T @ W64.T) for g in range(G)])
    wgs = wgs.astype(np.float32)

    # Per-core, per-group anchors: A(m, j) = T^(16384 m + 1024 j) @ X
    anchor_steps = []
    A = X
    for _ in range(NCORES * GROUPS):
        anchor_steps.append(A)
        A = TG @ A
    anchors_all = np.asarray(anchor_steps, np.float64).reshape(NCORES, GROUPS, S, S)
    anchors = [np.ascontiguousarray(anchors_all[m]).astype(np.float32)
               for m in range(NCORES)]
    return anchors, wgs


def kernel(sentence_len, pos_initial, pos_transition, W, b):
    from concourse.bass_utils import run_bass_kernel_spmd

    assert int(sentence_len) == L, f"kernel hardcodes L={L}, got {sentence_len}"
    b = np.asarray(b, np.float32)

    anchors, wgs = _host_precompute(pos_initial, pos_transition, W)

    nc = _build_program()
    in_maps = [{"anchors": anchors[m], "wgs": wgs} for m in range(NCORES)]
    res = run_bass_kernel_spmd(nc, in_maps, core_ids=list(range(NCORES)))
    full = np.concatenate([res.results[m]["out"] for m in range(NCORES)], axis=0)
    if np.any(b != 0):
        full = full + b[None, :]
    return full
